# revision 61
# baseline (speedup 1.0000x reference)
"""Trainium2 Bass kernel for nn_BaseGenerator (4-layer dense transformer).

Strategy: pure data-parallel over batch (B=8 -> 8 NeuronCores, no
collectives).  Each core runs the full transformer on one batch element.

Precision/speed scheme (cost model: bf16/fp16 matmul = 1 cycle/col,
fp8-e4m3 DoubleRow = 0.5 cycle/col with K=256 per instruction):
  - fp16 everywhere bf16 would be used (same speed, 4 more mantissa bits)
  - QKV, V, FFN1 GEMMs: fp8 DoubleRow with 3-term error compensation
    (w8*x8 + dw8*x8 + w8*dx8, dropping dw*dx) => 1.33x bf16 speed at
    ~fp16 accuracy.  Weights pre-scaled by 2^8 into e4m3's normal range.
  - attention AV: raw fp8 DoubleRow (at = exp(scores) quantized e4m3
    self-normalizes via the ones-row denominator; v quantized e4m3)
  - scores / out-proj / FFN2 / head: fp16 (activation-quant error there
    is too expensive);  additive mask applied by an fp8 DoubleRow
    identity matmul straight into the scores PSUM (masked = -240).
  - causal restriction: the scores matmul only covers q >= kc*128 (the
    mask matmul runs first with start=True and fills the rest with -240).

Layouts:
  - residual stream h: 8 fp16 tiles [128, S] feature-major
  - fp8 pair tiles [128, 2, S]: slot i = feature chunk 2j+i (DoubleRow
    [K,2,*] operands); dhp pair tiles hold the fp8 quantization residual
  - weights host-blocked into (w8, dw8) fp8 pairs / fp16 lhsT blocks
"""

import os
import sys

for _p in ("/opt/trn_rl_repo",):
    if _p not in sys.path:
        sys.path.insert(0, _p)

import ml_dtypes
import numpy as np

import concourse.bass as bass
import concourse.mybir as mybir
import concourse.tile as tile
from concourse import bacc
from concourse.bass_utils import run_bass_kernel_spmd

F16 = np.float16
F8 = ml_dtypes.float8_e4m3

L, E, H, F = 4, 1024, 16, 4096
B, S = 8, 512
VV, VR = 40, 30
DIST_V = 200
PAD_ID = 0
DH = E // H  # 64
NE = E // 128  # 8 feature chunks
NEP = NE // 2  # 4 fp8 pair tiles
NF = F // 128  # 32
NO = 10  # logit row tiles (1280 padded)
NEG = -240.0  # e4m3-representable "minus infinity" for additive mask

WSC = 256.0  # host-side weight scale before fp8 cast
DEQ = 1.0 / WSC

f32 = mybir.dt.float32
f16 = mybir.dt.float16
f8 = mybir.dt.float8e4
AF = mybir.ActivationFunctionType
OP = mybir.AluOpType
DR = mybir.MatmulPerfMode.DoubleRow

_CACHE = {}


# ----------------------------------------------------------------------------
# host-side input prep
# ----------------------------------------------------------------------------

def _f8(x):
    return np.ascontiguousarray(np.asarray(x, np.float32).astype(F8))


def _f16(x):
    return np.ascontiguousarray(np.asarray(x, np.float32).astype(F16))


def _block_dr2(W, gsize):
    """W [O, I] -> fp8 (w8, dw8) blocks [G, 128, I//256, 2(i), 2(t), gsize]
    with [g, p, cp, i, t, o] <- W[g*gsize + o, cp*256 + i*128 + p]*WSC,
    t=0: e4m3 quant, t=1: e4m3 residual."""
    O, I = W.shape
    Ws = np.asarray(W, np.float32) * WSC
    w8 = Ws.astype(F8).astype(np.float32)
    dw8 = (Ws - w8).astype(F8).astype(np.float32)
    out = np.empty((O // gsize, 128, I // 256, 2, 2, gsize), F8)
    for t, wv in enumerate((w8, dw8)):
        Wb = wv.reshape(O // gsize, gsize, I // 256, 2, 128)  # g o cp i p
        out[:, :, :, :, t, :] = Wb.transpose(0, 4, 2, 3, 1).astype(F8)
    return np.ascontiguousarray(out)


def _pp(v):  # [..., N*128] -> [..., 128, N]
    *lead, N = v.shape
    return np.ascontiguousarray(
        v.reshape(*lead, N // 128, 128).swapaxes(-1, -2).astype(np.float32)
    )


def _block_lhsT(W, gsize):
    # fp16 path: W [O, I] -> [G, 128, I//128, gsize]
    O, I = W.shape
    G = O // gsize
    nc_ = I // 128
    Wb = W.reshape(G, gsize, nc_, 128)
    Wb = np.moveaxis(Wb, -1, -3)
    Wb = np.swapaxes(Wb, -1, -2)
    return np.ascontiguousarray(Wb)


def _prep_shared(inp):
    out = {}

    Wqkv = np.asarray(inp["Wqkv"], np.float32).copy()  # [L, 3E, E]
    bqkv = np.asarray(inp["bqkv"], np.float32).copy()  # [L, 3E]
    # 1/sqrt(dh) is applied via the q copy-out scale; bq rows pre-divided.
    att_sc = 1.0 / np.sqrt(DH)
    bqkv[:, :E] *= att_sc

    # wqkv fp8 3-term blocks: [L, 6, 128, 4, 2, 2, 512] (Q g0-1, K g2-3, V g4-5)
    out["wqkv"] = np.stack([_block_dr2(Wqkv[l], 512) for l in range(L)])

    # wo fp16 head-pair lhsT: [L, 4(og), 128(hh*64+d), 8(hp), 256]
    Wo = np.asarray(inp["Wo"], np.float32)  # [L, E, E]
    woh = Wo.reshape(L, 4, 256, 8, 2, DH)   # l og o hp hh d
    woh = woh.transpose(0, 1, 4, 5, 3, 2)   # l og hh d hp o
    out["wo"] = _f16(woh.reshape(L, 4, 128, 8, 256))

    W1 = np.asarray(inp["W1"], np.float32)  # [L, F, E]
    out["w1"] = np.stack([_block_dr2(W1[l], 512) for l in range(L)])
    # w2 fp8 3-term: [L, 2(g), 4(cpg), 128, 4(cpi), 2, 2, 512]
    W2 = np.asarray(inp["W2"], np.float32)  # [L, E, F]
    w2b = np.stack([_block_dr2(W2[l], 512) for l in range(L)])
    # [L, 2, 128, 16, 2, 2, 512] -> split cp 16 -> (4, 4)
    w2b = w2b.reshape(L, 2, 128, 4, 4, 2, 2, 512).transpose(
        0, 1, 3, 2, 4, 5, 6, 7)
    out["w2"] = np.ascontiguousarray(w2b)

    # head GEMM commuted past the final LN:
    # logits = rstd*(x@Wp^T) - (mean*rstd)*rowsum(Wp) + (genW@lnf_b + gen_b)
    # with Wp = genW * lnf_s
    genW = np.asarray(inp["gen_W"], np.float32)  # [1200, E]
    lnf_s_v = np.asarray(inp["lnf_s"], np.float32)
    lnf_b_v = np.asarray(inp["lnf_b"], np.float32)
    Wp = genW * lnf_s_v[None, :]
    genW_pad = np.zeros((1280, E), np.float32)
    genW_pad[:1200] = Wp
    out["genw"] = _f16(_block_lhsT(genW_pad, 256))  # [5, 128, 8, 256]

    nws = np.zeros((1280,), np.float32)
    nws[:1200] = -Wp.sum(1)
    out["negw1sum_pp"] = np.ascontiguousarray(nws.reshape(NO, 128).T)
    gen_b = np.asarray(inp["gen_b"], np.float32)
    gbp = np.zeros((1280,), np.float32)
    gbp[:1200] = genW @ lnf_b_v + gen_b
    out["gen_b_pp"] = np.ascontiguousarray(gbp.reshape(NO, 128).T)  # [128, 10]

    out["bqkv_pp"] = _pp(bqkv[:, : 2 * E])  # [L, 128, 16] (q rows /8)
    out["bv_row"] = _f16(bqkv[:, 2 * E:].reshape(L, 1, E) * WSC)  # [L, 1, E]
    out["bo_pp"] = _pp(np.asarray(inp["bo"], np.float32))  # [L, 128, 8]
    out["b1_pp"] = _pp(np.asarray(inp["b1"], np.float32))  # [L, 128, 32]
    out["b2_pp"] = _pp(np.asarray(inp["b2"], np.float32))  # [L, 128, 8]

    ln_s = np.stack([np.asarray(inp["ln1_s"], np.float32),
                     np.asarray(inp["ln2_s"], np.float32)], 1)  # [L, 2, E]
    ln_b = np.stack([np.asarray(inp["ln1_b"], np.float32),
                     np.asarray(inp["ln2_b"], np.float32)], 1)
    out["ln_s_pp"] = _pp(ln_s)  # [L, 2, 128, 8]
    out["ln_b_pp"] = _pp(ln_b)
    out["lnf_s_pp"] = _pp(np.asarray(inp["lnf_s"], np.float32))  # [128, 8]
    out["lnf_b_pp"] = _pp(np.asarray(inp["lnf_b"], np.float32))

    out["valemb"] = _f16(np.asarray(inp["val_emb"], np.float32))   # [40, E]
    out["ringemb"] = _f16(np.asarray(inp["ring_emb"], np.float32))  # [30, E]

    # DR identity for mask add: idz[p, v, i, m] = 1 if i==v and p==m
    idz = np.zeros((128, 2, 2, 128), np.float32)
    for v in range(2):
        idz[:, v, v, :] = np.eye(128)
    out["idz"] = _f8(idz)
    out["ones_row"] = _f16(np.ones((1, S), np.float32))
    out["iota_col"] = np.ascontiguousarray(
        np.arange(128, dtype=np.float32).reshape(128, 1))
    out["ones_col"] = _f16(np.ones((128, 1), np.float32))
    return out


def _prep_percore(inp):
    """Per-core tensors: token rows + additive attention mask (fp8)."""
    val = np.asarray(inp["val_sequences"]).astype(np.int64)    # [B, S]
    ring = np.asarray(inp["ring_sequences"]).astype(np.int64)  # [B, S]
    dist = np.asarray(inp["distance_squares"]).astype(np.int64)  # [B, S, S]
    de = np.asarray(inp["dist_emb"], np.float32)  # [200, H]

    # mask[b, h, k, q] = de[dist[b, q, k], h] or NEG
    m = de[dist]                         # [B, S(q), S(k), H]
    m = m.transpose(0, 3, 2, 1)          # [B, H, k, q]
    kk = np.arange(S)
    causal = kk[:, None] <= kk[None, :]  # [k, q] keep where k <= q
    m = np.where(causal[None, None], m, NEG)
    padk = val == PAD_ID  # [B, S]
    m = np.where(padk[:, None, :, None], NEG, m)
    # -> [B, H, 128(p), 4(kc), S(q)] with k = kc*128 + p
    m = m.reshape(B, H, 4, 128, S).transpose(0, 1, 3, 2, 4)
    m = np.ascontiguousarray(m.reshape(B, H, 128, 4 * S).astype(F8))

    cores = []
    for b in range(B):
        cores.append({
            "mask": m[b],
            "valrow": np.ascontiguousarray(val[b].reshape(1, S).astype(F16)),
            "ringrow": np.ascontiguousarray(ring[b].reshape(1, S).astype(F16)),
        })
    return cores


# ----------------------------------------------------------------------------
# device program
# ----------------------------------------------------------------------------

def _declare(nc):
    d = {}

    def di(name, shape, dt):
        d[name] = nc.dram_tensor(name, list(shape), dt, kind="ExternalInput").ap()

    di("wqkv", (L, 6, 128, 4, 2, 2, 512), f8)
    di("wo", (L, 4, 128, 8, 256), f16)
    di("w1", (L, 8, 128, 4, 2, 2, 512), f8)
    di("w2", (L, 2, 4, 128, 4, 2, 2, 512), f8)
    di("genw", (5, 128, 8, 256), f16)
    di("gen_b_pp", (128, NO), f32)
    di("negw1sum_pp", (128, NO), f32)
    di("bqkv_pp", (L, 128, 16), f32)
    di("bv_row", (L, 1, E), f16)
    di("bo_pp", (L, 128, 8), f32)
    di("b1_pp", (L, 128, 32), f32)
    di("b2_pp", (L, 128, 8), f32)
    di("ln_s_pp", (L, 2, 128, 8), f32)
    di("ln_b_pp", (L, 2, 128, 8), f32)
    di("lnf_s_pp", (128, 8), f32)
    di("lnf_b_pp", (128, 8), f32)
    di("valemb", (VV, E), f16)
    di("ringemb", (VR, E), f16)
    di("idz", (128, 2, 2, 128), f8)
    di("ones_row", (1, S), f16)
    di("iota_col", (128, 1), f32)
    di("ones_col", (128, 1), f16)
    di("mask", (H, 128, 4 * S), f8)
    di("valrow", (1, S), f16)
    di("ringrow", (1, S), f16)
    d["logits"] = nc.dram_tensor(
        "logits", [NO, 128, S], f32, kind="ExternalOutput"
    ).ap()
    if os.environ.get("BG_DEBUG"):
        def do(name, shape, dt=f16):
            d[name] = nc.dram_tensor(name, list(shape), dt,
                                     kind="ExternalOutput").ap()
        do("dbg_h0", (128, S))
        do("dbg_hp0", (128, 2, S), f8)
        do("dbg_dhp0", (128, 2, S), f8)
        do("dbg_qk", (128, S))
        do("dbg_v", (128, 2, H, DH + 1), f8)
        do("dbg_at", (128, 2 * S), f8)
        do("dbg_ctx", (128, S))
        do("dbg_r1", (128, S))
        do("dbg_h1", (128, S))
        do("dbg_ff", (128, S))
        do("dbg_r2", (128, S))
    return d


def _emit(nc, tc, d, ctx):
    mm = nc.tensor.matmul

    cpool = ctx.enter_context(tc.tile_pool(name="cpool", bufs=1))
    maskpool = ctx.enter_context(tc.tile_pool(name="maskpool", bufs=3))
    wpool = ctx.enter_context(tc.tile_pool(name="wpool", bufs=3))
    wopool = ctx.enter_context(tc.tile_pool(name="wopool", bufs=2))
    hpool = ctx.enter_context(tc.tile_pool(name="hpool", bufs=16))
    hppool = ctx.enter_context(tc.tile_pool(name="hppool", bufs=8))
    dhppool = ctx.enter_context(tc.tile_pool(name="dhppool", bufs=8))
    qkpool = ctx.enter_context(tc.tile_pool(name="qkpool", bufs=16))
    vpool = ctx.enter_context(tc.tile_pool(name="vpool", bufs=3))
    atpool = ctx.enter_context(tc.tile_pool(name="atpool", bufs=4))
    ctxpool = ctx.enter_context(tc.tile_pool(name="ctxpool", bufs=9))
    ffpool = ctx.enter_context(tc.tile_pool(name="ffpool", bufs=17))
    tmppool = ctx.enter_context(tc.tile_pool(name="tmppool", bufs=6))
    smallf = ctx.enter_context(tc.tile_pool(name="smallf", bufs=3))
    smallb = ctx.enter_context(tc.tile_pool(name="smallb", bufs=3))
    recpool = ctx.enter_context(tc.tile_pool(name="recpool", bufs=3))
    outpool = ctx.enter_context(tc.tile_pool(name="outpool", bufs=2))
    pppool = ctx.enter_context(tc.tile_pool(name="pppool", bufs=4))

    ps_gemm = ctx.enter_context(tc.tile_pool(name="ps_gemm", bufs=4, space="PSUM"))
    ps_wide = ctx.enter_context(tc.tile_pool(name="ps_wide", bufs=2, space="PSUM"))

    hw = nc.sync  # HWDGE dma engine

    # --- constants -----------------------------------------------------------
    idz = cpool.tile([128, 2, 2, 128], f8)
    hw.dma_start(out=idz, in_=d["idz"])
    ones_row = cpool.tile([1, S], f16)
    hw.dma_start(out=ones_row, in_=d["ones_row"])
    iota_col = cpool.tile([128, 1], f32)
    hw.dma_start(out=iota_col, in_=d["iota_col"])
    ones_col = cpool.tile([128, 1], f16)
    hw.dma_start(out=ones_col, in_=d["ones_col"])
    valemb = cpool.tile([VV, E], f16)
    hw.dma_start(out=valemb, in_=d["valemb"])
    ringemb = cpool.tile([VR, E], f16)
    hw.dma_start(out=ringemb, in_=d["ringemb"])
    genb_pp = cpool.tile([128, NO], f32)
    hw.dma_start(out=genb_pp, in_=d["gen_b_pp"])
    negw1s = cpool.tile([128, NO], f32)
    hw.dma_start(out=negw1s, in_=d["negw1sum_pp"])
    eps_t = cpool.tile([128, 1], f32)
    nc.vector.memset(eps_t, 1e-5)
    actwarm = cpool.tile([1, 1], f32)
    lnf_s = cpool.tile([128, 8], f32)
    hw.dma_start(out=lnf_s, in_=d["lnf_s_pp"])
    lnf_b = cpool.tile([128, 8], f32)
    hw.dma_start(out=lnf_b, in_=d["lnf_b_pp"])

    # --- embedding -----------------------------------------------------------
    with nc.named_scope("embed"):
        valR = tmppool.tile([VV, S], f16, tag="ffh", bufs=3)
        nc.gpsimd.dma_start(out=valR, in_=d["valrow"].to_broadcast((VV, S)))
        ringR = tmppool.tile([VR, S], f16, tag="ffh", bufs=3)
        nc.gpsimd.dma_start(out=ringR, in_=d["ringrow"].to_broadcast((VR, S)))
        oh_val = tmppool.tile([VV, S], f16, tag="ffh", bufs=3)
        nc.vector.tensor_scalar(oh_val, valR, iota_col[:VV, :], None, OP.is_equal)
        oh_ring = tmppool.tile([VR, S], f16, tag="ffh", bufs=3)
        nc.vector.tensor_scalar(oh_ring, ringR, iota_col[:VR, :], None, OP.is_equal)

        h_t = []
        hp_t = [hppool.tile([128, 2, S], f8, tag="hp", name=f"emb_hp{j}")
                for j in range(NEP)]
        dhp_t = [dhppool.tile([128, 2, S], f8, tag="dhp", name=f"emb_dhp{j}")
                 for j in range(NEP)]
        for c in range(NE):
            ps = ps_gemm.tile([128, S], f32, tag="gemm")
            mm(ps, valemb[:, c * 128:(c + 1) * 128], oh_val, start=True, stop=False)
            mm(ps, ringemb[:, c * 128:(c + 1) * 128], oh_ring, start=False, stop=True)
            ht = hpool.tile([128, S], f16, tag="h")
            nc.scalar.activation(ht, ps, AF.Copy, scale=float(np.sqrt(E)))
            nc.scalar.activation(hp_t[c // 2][:, c % 2, :], ps, AF.Copy,
                                 scale=float(np.sqrt(E)))
            nc.vector.tensor_sub(dhp_t[c // 2][:, c % 2, :], ht,
                                 hp_t[c // 2][:, c % 2, :])
            h_t.append(ht)
        if "dbg_h0" in d:
            hw.dma_start(out=d["dbg_h0"], in_=h_t[0])
            hw.dma_start(out=d["dbg_hp0"], in_=hp_t[0])
            hw.dma_start(out=d["dbg_dhp0"], in_=dhp_t[0])

    # --- layers --------------------------------------------------------------
    env = dict(locals())
    for l in range(L):
        h_t, hp_t, dhp_t = _layer(nc, tc, d, l, h_t, hp_t, dhp_t, env)

    # --- final LN (stats only) + head on pre-LN x ----------------------------
    with nc.named_scope("final"):
        genw_sb = []
        for g in range(5):
            wt = wopool.tile([128, 8, 256], f16, tag="genw", bufs=3)
            hw.dma_start(out=wt, in_=d["genw"][g])
            genw_sb.append(wt)
        tmppool = env["tmppool"]; smallf = env["smallf"]; smallb = env["smallb"]
        recpool = env["recpool"]
        sums_r = ps_gemm.tile([1, S], f32, tag="gemm", name="flnsum_r")
        sums_q = ps_gemm.tile([1, S], f32, tag="gemm", name="flnsum_q")
        sq_t = []
        for c in range(NE):
            sq = tmppool.tile([128, S], f16, tag="sq", bufs=8)
            nc.vector.tensor_mul(sq, h_t[c], h_t[c])
            sq_t.append(sq)
        for c in range(NE):
            mm(sums_r, ones_col, h_t[c], start=(c == 0), stop=(c == NE - 1))
        for c in range(NE):
            mm(sums_q, ones_col, sq_t[c], start=(c == 0), stop=(c == NE - 1))
        s2 = smallf.tile([1, S], f32, tag="sf")
        nc.scalar.activation(s2, sums_r, AF.Square)
        varE = smallf.tile([1, S], f32, tag="sf")
        nc.vector.scalar_tensor_tensor(varE, s2, -1.0 / E, sums_q,
                                       OP.mult, OP.add)
        std = smallf.tile([1, S], f32, tag="sf")
        nc.scalar.activation(std, varE, AF.Sqrt, bias=eps_t[:1, :],
                             scale=1.0 / E)
        rstd = smallf.tile([1, S], f32, tag="sf")
        nc.vector.reciprocal_approx_fast(out=rstd, in_=std)
        rstd_b = smallb.tile([1, S], f16, tag="sb")
        nc.vector.tensor_copy(rstd_b, rstd)
        mrstd_b = smallb.tile([1, S], f16, tag="sb")
        # mean * rstd
        nc.vector.scalar_tensor_tensor(mrstd_b, sums_r, 1.0 / E, rstd,
                                       OP.mult, OP.mult)
        rstdR = recpool.tile([128, S], f16, tag="rec", name="flnrstdR")
        nc.gpsimd.partition_broadcast(rstdR, rstd_b, channels=128)
        mrstdR = recpool.tile([128, S], f16, tag="rec", name="flnmrstdR")
        nc.gpsimd.partition_broadcast(mrstdR, mrstd_b, channels=128)
        for mt in range(NO):
            g, mi = divmod(mt, 2)
            ps = ps_gemm.tile([128, S], f32, tag="gemm")
            for c in range(NE):
                mm(ps, genw_sb[g][:, c, mi * 128:(mi + 1) * 128], h_t[c],
                   start=(c == 0), stop=(c == NE - 1))
            gt = tmppool.tile([128, S], f16, tag="tmp", bufs=3,
                              name=f"gt{mt}")
            nc.vector.tensor_mul(gt, ps, rstdR)
            g2 = tmppool.tile([128, S], f16, tag="ffh", bufs=3,
                              name=f"g2{mt}")
            nc.vector.scalar_tensor_tensor(g2, mrstdR, negw1s[:, mt:mt + 1],
                                           gt, OP.mult, OP.add)
            ot = outpool.tile([128, S], f32, tag="f32out")
            nc.scalar.activation(ot, g2, AF.Identity,
                                 bias=genb_pp[:, mt:mt + 1])
            hw.dma_start(out=d["logits"][mt], in_=ot)


def _layernorm(nc, r_t, s_pp, b_pp, env, want_fp8=True, tagsuf=""):
    """r_t: 8 fp16 [128, S] tiles -> (fp16 tiles, fp8 pair tiles, fp8
    residual pair tiles).  Chunk sums tree-reduce on DVE, partition
    reduction via two ones-column matmuls per statistic."""
    mm = nc.tensor.matmul
    tmppool = env["tmppool"]; smallf = env["smallf"]; smallb = env["smallb"]
    recpool = env["recpool"]; hpool = env["hpool"]
    hppool = env["hppool"]; dhppool = env["dhppool"]
    ps_gemm = env["ps_gemm"]; ones_col = env["ones_col"]; eps_t = env["eps_t"]

    sums_r = ps_gemm.tile([1, S], f32, tag="gemm", name="lnsum_r" + tagsuf)
    sums_q = ps_gemm.tile([1, S], f32, tag="gemm", name="lnsum_q" + tagsuf)
    sq_t = []
    for c in range(NE):
        sq = tmppool.tile([128, S], f16, tag="sq", bufs=8)
        nc.vector.tensor_mul(sq, r_t[c], r_t[c])
        sq_t.append(sq)
    for c in range(NE):
        mm(sums_r, ones_col, r_t[c], start=(c == 0), stop=(c == NE - 1))
    for c in range(NE):
        mm(sums_q, ones_col, sq_t[c], start=(c == 0), stop=(c == NE - 1))

    # mean broadcast early; r centered while the var chain runs
    mean_b = smallb.tile([1, S], f16, tag="sb")
    nc.vector.tensor_scalar(mean_b, sums_r, 1.0 / E, None, OP.mult)
    meanR = recpool.tile([128, S], f16, tag="rec", name="lnmeanR" + tagsuf)
    nc.gpsimd.partition_broadcast(meanR, mean_b, channels=128)
    t2_t = []
    for c in range(NE):
        t2 = tmppool.tile([128, S], f16, tag="sq", bufs=8)
        eng = nc.vector if c < 6 else nc.gpsimd
        eng.tensor_sub(t2, r_t[c], meanR)
        t2_t.append(t2)

    s2 = smallf.tile([1, S], f32, tag="sf")
    nc.scalar.activation(s2, sums_r, AF.Square)
    varE = smallf.tile([1, S], f32, tag="sf")
    # varE = sumsq - s2/E  (= E * var)
    nc.vector.scalar_tensor_tensor(varE, s2, -1.0 / E, sums_q, OP.mult, OP.add)
    std = smallf.tile([1, S], f32, tag="sf")
    nc.scalar.activation(std, varE, AF.Sqrt, bias=eps_t[:1, :], scale=1.0 / E)
    rstd = smallf.tile([1, S], f32, tag="sf")
    nc.vector.reciprocal_approx_fast(out=rstd, in_=std)
    rstd_b = smallb.tile([1, S], f16, tag="sb")
    nc.vector.tensor_copy(rstd_b, rstd)
    rstdR = recpool.tile([128, S], f16, tag="rec", name="lnrstdR" + tagsuf)
    nc.gpsimd.partition_broadcast(rstdR, rstd_b, channels=128)

    out_t = []
    outp_t = [hppool.tile([128, 2, S], f8, tag="hp", name=f"hp{tagsuf}_{j}")
              for j in range(NEP)] if want_fp8 else None
    outdp_t = [dhppool.tile([128, 2, S], f8, tag="dhp", name=f"dhp{tagsuf}_{j}")
               for j in range(NEP)] if want_fp8 else None
    for c in range(NE):
        eng = nc.vector if c < 6 else nc.gpsimd
        t1 = tmppool.tile([128, S], f16, tag="tmp", bufs=3)
        eng.tensor_mul(t1, t2_t[c], rstdR)
        ht = hpool.tile([128, S], f16, tag="h")
        eng.tensor_scalar(ht, t1, s_pp[:, c:c + 1], b_pp[:, c:c + 1],
                          OP.mult, OP.add)
        out_t.append(ht)
        if want_fp8:
            nc.scalar.activation(outp_t[c // 2][:, c % 2, :], t1, AF.Identity,
                                 bias=b_pp[:, c:c + 1], scale=s_pp[:, c:c + 1])
            nc.vector.tensor_sub(outdp_t[c // 2][:, c % 2, :], ht,
                                 outp_t[c // 2][:, c % 2, :])
    return out_t, outp_t, outdp_t


def _layer(nc, tc, d, l, h_t, hp_t, dhp_t, env):
    mm = nc.tensor.matmul
    hw = nc.sync
    wpool = env["wpool"]; wopool = env["wopool"]; hpool = env["hpool"]
    qkpool = env["qkpool"]; vpool = env["vpool"]; atpool = env["atpool"]
    ctxpool = env["ctxpool"]; ffpool = env["ffpool"]; tmppool = env["tmppool"]
    smallf = env["smallf"]; recpool = env["recpool"]; pppool = env["pppool"]
    maskpool = env["maskpool"]
    ps_gemm = env["ps_gemm"]; ps_wide = env["ps_wide"]
    ones_row = env["ones_row"]; idz = env["idz"]

    # per-layer small params
    bqkv_pp = pppool.tile([128, 16], f32, tag="pp16")
    hw.dma_start(out=bqkv_pp, in_=d["bqkv_pp"][l])
    bv_row = pppool.tile([1, E], f16, tag="bvrow", bufs=1)
    hw.dma_start(out=bv_row, in_=d["bv_row"][l])
    bo_pp = pppool.tile([128, 8], f32, tag="pp8")
    hw.dma_start(out=bo_pp, in_=d["bo_pp"][l])
    b1_pp = pppool.tile([128, 32], f32, tag="pp32")
    hw.dma_start(out=b1_pp, in_=d["b1_pp"][l])
    b2_pp = pppool.tile([128, 8], f32, tag="pp8")
    hw.dma_start(out=b2_pp, in_=d["b2_pp"][l])
    ln_s = [pppool.tile([128, 8], f32, tag="pp8", name=f"lns{l}_{i}")
            for i in range(2)]
    ln_b = [pppool.tile([128, 8], f32, tag="pp8", name=f"lnb{l}_{i}")
            for i in range(2)]
    for i in range(2):
        hw.dma_start(out=ln_s[i], in_=d["ln_s_pp"][l, i])
        hw.dma_start(out=ln_b[i], in_=d["ln_b_pp"][l, i])

    def gemm3(ps, wt, mi, xp, dxp, ncp=4):
        """3-term compensated fp8 DR accumulation into ps."""
        for cp in range(ncp):
            w8 = wt[:, cp, :, 0, mi * 128:(mi + 1) * 128]
            dw8 = wt[:, cp, :, 1, mi * 128:(mi + 1) * 128]
            mm(ps, w8, xp[cp], start=(cp == 0), stop=False, perf_mode=DR)
            mm(ps, dw8, xp[cp], start=False, stop=False, perf_mode=DR)
            mm(ps, w8, dxp[cp], start=False, stop=(cp == ncp - 1), perf_mode=DR)

    # --- QKV -----------------------------------------------------------------
    with nc.named_scope(f"L{l}_qkv"):
        qk_t = []  # 16 fp16 tiles: q 0..7, k 8..15
        for g in range(4):  # Q, K feature-major, fp8 3-term
            wt = wpool.tile([128, 4, 2, 2, 512], f8, tag="w")
            hw.dma_start(out=wt, in_=d["wqkv"][l, g])
            for mi in range(4):
                mt = g * 4 + mi
                ps = ps_gemm.tile([128, S], f32, tag="gemm")
                gemm3(ps, wt, mi, hp_t, dhp_t)
                qk = qkpool.tile([128, S], f16, tag="qk")
                sc = DEQ / np.sqrt(DH) if g < 2 else DEQ
                nc.scalar.activation(qk, ps, AF.Identity,
                                     bias=bqkv_pp[:, mt:mt + 1], scale=float(sc))
                if l == 0 and mt == 0 and "dbg_qk" in d:
                    hw.dma_start(out=d["dbg_qk"], in_=qk)
                qk_t.append(qk)
        # V token-major fp8 pair tiles [128, 2, H, DH+1] + fp8 residual
        v_t = []
        dv_t = []
        for kcp in range(2):
            vt = vpool.tile([128, 2, H, DH + 1], f8, tag="v")
            nc.vector.memset(vt[:, :, :, DH:DH + 1], 1.0)
            v_t.append(vt)
            dvt = vpool.tile([128, 2, H, DH + 1], f8, tag="dv", bufs=2)
            nc.vector.memset(dvt[:, :, :, DH:DH + 1], 0.0)
            dv_t.append(dvt)
        for g in range(2):
            wt = wpool.tile([128, 4, 2, 2, 512], f8, tag="w")
            hw.dma_start(out=wt, in_=d["wqkv"][l, 4 + g])
            for n in range(4):
                ps = ps_gemm.tile([128, S], f32, tag="gemm")
                for cp in range(4):
                    xs = hp_t[cp][:, :, n * 128:(n + 1) * 128]
                    dxs = dhp_t[cp][:, :, n * 128:(n + 1) * 128]
                    w8 = wt[:, cp, :, 0, :]
                    dw8 = wt[:, cp, :, 1, :]
                    mm(ps, xs, w8, start=(cp == 0), stop=False, perf_mode=DR)
                    mm(ps, xs, dw8, start=False, stop=False, perf_mode=DR)
                    mm(ps, dxs, w8, start=False, stop=False, perf_mode=DR)
                mm(ps, ones_row[:, :128], bv_row[:, g * 512:(g + 1) * 512],
                   start=False, stop=True)
                v8s = v_t[n // 2][:, n % 2, g * 8:(g + 1) * 8, 0:DH]
                nc.scalar.activation(
                    v8s, ps.rearrange("p (a b) -> p a b", a=8),
                    AF.Copy, scale=DEQ)
                nc.vector.scalar_tensor_tensor(
                    dv_t[n // 2][:, n % 2, g * 8:(g + 1) * 8, 0:DH],
                    ps.rearrange("p (a b) -> p a b", a=8), DEQ, v8s,
                    OP.mult, OP.subtract)
        if l == 0 and "dbg_v" in d:
            hw.dma_start(out=d["dbg_v"], in_=v_t[0])

    # --- attention ------------------------------------------------------------
    with nc.named_scope(f"L{l}_attn"):
        ctx_t = [ctxpool.tile([128, S], f16, tag="ctx", name=f"cx{l}_{j}")
                 for j in range(8)]
        at_q = {}

        def emit_scores(h):
            qt = qk_t[h // 2]
            kt = qk_t[8 + h // 2]
            r0 = (h % 2) * DH
            mt_ = maskpool.tile([128, 4 * S], f8, tag="mask", name=f"mk{l}_{h}")
            hw.dma_start(out=mt_, in_=d["mask"][h])
            for kcp in range(2):
                sps = ps_wide.tile([128, 2 * S], f32, tag="wide",
                                   name=f"s{l}_{h}_{kcp}")
                for kci in range(2):
                    kc = kcp * 2 + kci
                    qlo = kc * 128
                    # mask first (start=True fills masked region with -240)
                    mm(sps[:, kci * S:(kci + 1) * S], idz[:, kci],
                       mt_[:, kcp * 2 * S:(kcp + 1) * 2 * S].rearrange(
                           "p (two n) -> p two n", two=2),
                       start=True, stop=False, perf_mode=DR)
                    # causal-restricted scores accumulate
                    mm(sps[:, kci * S + qlo:(kci + 1) * S],
                       kt[r0:r0 + DH, kc * 128:(kc + 1) * 128],
                       qt[r0:r0 + DH, qlo:], start=False, stop=True)
                at = atpool.tile([128, 2 * S], f8, tag="at",
                                 name=f"a{l}_{h}_{kcp}")
                nc.scalar.activation(at, sps, AF.Exp)
                if l == 0 and h == 0 and kcp == 0 and "dbg_at" in d:
                    hw.dma_start(out=d["dbg_at"], in_=at)
                at_q[(h, kcp)] = at

        def emit_av(h):
            cps = ps_gemm.tile([128, S], f32, tag="gemm", name=f"c{l}_{h}")
            for kcp in range(2):
                at = at_q.pop((h, kcp))
                atr = at.rearrange("p (two n) -> p two n", two=2)
                mm(cps[0:DH + 1, :], v_t[kcp][:, :, h, :], atr,
                   start=(kcp == 0), stop=False, perf_mode=DR)
                mm(cps[0:DH + 1, :], dv_t[kcp][:, :, h, :], atr,
                   start=False, stop=(kcp == 1), perf_mode=DR)
            srow = smallf.tile([1, S], f32, tag="sf", name=f"sr{l}_{h}")
            nc.vector.tensor_copy(srow, cps[DH:DH + 1, :])
            rec = smallf.tile([1, S], f32, tag="sf", name=f"re{l}_{h}")
            nc.vector.reciprocal_approx_fast(out=rec, in_=srow)
            recR = recpool.tile([DH, S], f32, tag="recf", name=f"rr{l}_{h}")
            nc.gpsimd.partition_broadcast(recR, rec, channels=DH)
            hh = h % 2
            nc.vector.tensor_mul(ctx_t[h // 2][hh * DH:(hh + 1) * DH, :],
                                 cps[0:DH, :], recR)

        emit_scores(0)
        for h in range(1, H):
            emit_scores(h)
            emit_av(h - 1)
        emit_av(H - 1)

        # out-proj fp16 (K = 8 head-pair chunks) + residual
        r1_t = []
        for og in range(4):
            wt = wopool.tile([128, 8, 256], f16, tag="wo")
            hw.dma_start(out=wt, in_=d["wo"][l, og])
            for mi in range(2):
                mt = og * 2 + mi
                ps = ps_gemm.tile([128, S], f32, tag="gemm")
                for hp in range(8):
                    mm(ps, wt[:, hp, mi * 128:(mi + 1) * 128], ctx_t[hp],
                       start=(hp == 0), stop=(hp == 7))
                r1 = hpool.tile([128, S], f16, tag="h")
                nc.vector.scalar_tensor_tensor(r1, ps, bo_pp[:, mt:mt + 1],
                                               h_t[mt], OP.add, OP.add)
                if l == 0 and mt == 0 and "dbg_r1" in d:
                    hw.dma_start(out=d["dbg_ctx"], in_=ctx_t[0])
                    hw.dma_start(out=d["dbg_r1"], in_=r1)
                r1_t.append(r1)

    with nc.named_scope(f"L{l}_ln1"):
        h1_t, h1p_t, dh1p_t = _layernorm(nc, r1_t, ln_s[0], ln_b[0], env,
                                         tagsuf=f"1_{l}")
        if l == 0 and "dbg_h1" in d:
            hw.dma_start(out=d["dbg_h1"], in_=h1_t[0])

    # --- FFN -----------------------------------------------------------------
    with nc.named_scope(f"L{l}_ffn"):
        ffp_t = [ffpool.tile([128, 2, S], f8, tag="ff", name=f"ff{l}_{j}")
                 for j in range(NF // 2)]
        dffp_t = [ffpool.tile([128, 2, S], f8, tag="dff", name=f"dff{l}_{j}")
                  for j in range(NF // 2)]
        for g in range(8):
            wt = wpool.tile([128, 4, 2, 2, 512], f8, tag="w")
            hw.dma_start(out=wt, in_=d["w1"][l, g])
            for mi in range(4):
                mt = g * 4 + mi
                ps = ps_gemm.tile([128, S], f32, tag="gemm")
                gemm3(ps, wt, mi, h1p_t, dh1p_t)
                ft = tmppool.tile([128, S], f16, tag="ffh", bufs=3,
                                  name=f"ffh{l}_{mt}")
                nc.scalar.activation(ft, ps, AF.Gelu,
                                     bias=b1_pp[:, mt:mt + 1], scale=DEQ)
                f8s = ffp_t[mt // 2][:, mt % 2, :]
                nc.scalar.activation(f8s, ft, AF.Copy)
                nc.vector.tensor_sub(dffp_t[mt // 2][:, mt % 2, :], ft, f8s)
        r2_t = [None] * NE
        for g in range(2):
            pss = [ps_gemm.tile([128, S], f32, tag="gemm",
                                name=f"ff2ps{l}_{g}_{i}") for i in range(4)]
            for cpg in range(4):
                wt = wpool.tile([128, 4, 2, 2, 512], f8, tag="w",
                                name=f"w2t{l}_{g}_{cpg}")
                hw.dma_start(out=wt, in_=d["w2"][l, g, cpg])
                for cpi in range(4):
                    cp = cpg * 4 + cpi
                    for mi in range(4):
                        w8 = wt[:, cpi, :, 0, mi * 128:(mi + 1) * 128]
                        dw8 = wt[:, cpi, :, 1, mi * 128:(mi + 1) * 128]
                        mm(pss[mi], w8, ffp_t[cp], start=(cp == 0),
                           stop=False, perf_mode=DR)
                        mm(pss[mi], dw8, ffp_t[cp], start=False,
                           stop=False, perf_mode=DR)
                        mm(pss[mi], w8, dffp_t[cp], start=False,
                           stop=(cp == 15), perf_mode=DR)
            for mi in range(4):
                mt = g * 4 + mi
                f2o = tmppool.tile([128, S], f16, tag="ffh", bufs=3,
                                   name=f"f2o{l}_{mt}")
                nc.scalar.activation(f2o, pss[mi], AF.Identity,
                                     bias=b2_pp[:, mt:mt + 1], scale=DEQ)
                r2 = hpool.tile([128, S], f16, tag="h")
                nc.vector.tensor_add(r2, f2o, h1_t[mt])
                if l == 0 and mt == 0 and "dbg_r2" in d:
                    hw.dma_start(out=d["dbg_r2"], in_=r2)
                r2_t[mt] = r2

    with nc.named_scope(f"L{l}_ln2"):
        h2_t, h2p_t, dh2p_t = _layernorm(nc, r2_t, ln_s[1], ln_b[1], env,
                                         tagsuf=f"2_{l}")
    return h2_t, h2p_t, dh2p_t


def _build():
    if "nc" in _CACHE:
        return _CACHE["nc"]
    from contextlib import ExitStack

    nc = bacc.Bacc("TRN2", debug=False)
    d = _declare(nc)
    with tile.TileContext(nc) as tc:
        with ExitStack() as ctx:
            _emit(nc, tc, d, ctx)
    nc.compile()
    _CACHE["nc"] = nc
    return nc


def kernel_internal(inputs, trace=False, trace_kwargs=None):
    shared = _prep_shared(inputs)
    cores = _prep_percore(inputs)
    nc = _build()
    in_maps = []
    for b in range(B):
        m = dict(shared)
        m.update(cores[b])
        in_maps.append(m)
    res = run_bass_kernel_spmd(
        nc, in_maps, core_ids=list(range(B)), trace=trace,
        **(trace_kwargs or {}),
    )
    outs = []
    for b in range(B):
        lo = res.results[b]["logits"]  # [10, 128, 512]
        lo = lo.reshape(NO * 128, S)[:VV * VR].T  # [512, 1200]
        outs.append(lo)
    out = np.stack(outs).astype(np.float32)  # [B, S, 1200]
    return out, res


def kernel(**inputs):
    out, _ = kernel_internal(inputs)
    return out


# revision 68
# speedup vs baseline: 1.0073x; 1.0073x over previous
"""Trainium2 Bass kernel for nn_BaseGenerator (4-layer dense transformer).

Strategy: pure data-parallel over batch (B=8 -> 8 NeuronCores, no
collectives).  Each core runs the full transformer on one batch element.

Precision/speed scheme (cost model: bf16/fp16 matmul = 1 cycle/col,
fp8-e4m3 DoubleRow = 0.5 cycle/col with K=256 per instruction):
  - fp16 everywhere bf16 would be used (same speed, 4 more mantissa bits)
  - QKV, V, FFN1 GEMMs: fp8 DoubleRow with 3-term error compensation
    (w8*x8 + dw8*x8 + w8*dx8, dropping dw*dx) => 1.33x bf16 speed at
    ~fp16 accuracy.  Weights pre-scaled by 2^8 into e4m3's normal range.
  - attention AV: raw fp8 DoubleRow (at = exp(scores) quantized e4m3
    self-normalizes via the ones-row denominator; v quantized e4m3)
  - scores / out-proj / FFN2 / head: fp16 (activation-quant error there
    is too expensive);  additive mask applied by an fp8 DoubleRow
    identity matmul straight into the scores PSUM (masked = -240).
  - causal restriction: the scores matmul only covers q >= kc*128 (the
    mask matmul runs first with start=True and fills the rest with -240).

Layouts:
  - residual stream h: 8 fp16 tiles [128, S] feature-major
  - fp8 pair tiles [128, 2, S]: slot i = feature chunk 2j+i (DoubleRow
    [K,2,*] operands); dhp pair tiles hold the fp8 quantization residual
  - weights host-blocked into (w8, dw8) fp8 pairs / fp16 lhsT blocks
"""

import os
import sys

for _p in ("/opt/trn_rl_repo",):
    if _p not in sys.path:
        sys.path.insert(0, _p)

import ml_dtypes
import numpy as np

import concourse.bass as bass
import concourse.mybir as mybir
import concourse.tile as tile
from concourse import bacc
from concourse.bass_utils import run_bass_kernel_spmd

F16 = np.float16
F8 = ml_dtypes.float8_e4m3

L, E, H, F = 4, 1024, 16, 4096
B, S = 8, 512
VV, VR = 40, 30
DIST_V = 200
PAD_ID = 0
DH = E // H  # 64
NE = E // 128  # 8 feature chunks
NEP = NE // 2  # 4 fp8 pair tiles
NF = F // 128  # 32
NO = 10  # logit row tiles (1280 padded)
NEG = -240.0  # e4m3-representable "minus infinity" for additive mask

WSC = 256.0  # host-side weight scale before fp8 cast
DEQ = 1.0 / WSC

f32 = mybir.dt.float32
f16 = mybir.dt.float16
f8 = mybir.dt.float8e4
AF = mybir.ActivationFunctionType
OP = mybir.AluOpType
DR = mybir.MatmulPerfMode.DoubleRow

_CACHE = {}


# ----------------------------------------------------------------------------
# host-side input prep
# ----------------------------------------------------------------------------

def _f8(x):
    return np.ascontiguousarray(np.asarray(x, np.float32).astype(F8))


def _f16(x):
    return np.ascontiguousarray(np.asarray(x, np.float32).astype(F16))


def _block_dr2(W, gsize):
    """W [O, I] -> fp8 (w8, dw8) blocks [G, 128, I//256, 2(i), 2(t), gsize]
    with [g, p, cp, i, t, o] <- W[g*gsize + o, cp*256 + i*128 + p]*WSC,
    t=0: e4m3 quant, t=1: e4m3 residual."""
    O, I = W.shape
    Ws = np.asarray(W, np.float32) * WSC
    w8 = Ws.astype(F8).astype(np.float32)
    dw8 = (Ws - w8).astype(F8).astype(np.float32)
    out = np.empty((O // gsize, 128, I // 256, 2, 2, gsize), F8)
    for t, wv in enumerate((w8, dw8)):
        Wb = wv.reshape(O // gsize, gsize, I // 256, 2, 128)  # g o cp i p
        out[:, :, :, :, t, :] = Wb.transpose(0, 4, 2, 3, 1).astype(F8)
    return np.ascontiguousarray(out)


def _pp(v):  # [..., N*128] -> [..., 128, N]
    *lead, N = v.shape
    return np.ascontiguousarray(
        v.reshape(*lead, N // 128, 128).swapaxes(-1, -2).astype(np.float32)
    )


def _block_lhsT(W, gsize):
    # fp16 path: W [O, I] -> [G, 128, I//128, gsize]
    O, I = W.shape
    G = O // gsize
    nc_ = I // 128
    Wb = W.reshape(G, gsize, nc_, 128)
    Wb = np.moveaxis(Wb, -1, -3)
    Wb = np.swapaxes(Wb, -1, -2)
    return np.ascontiguousarray(Wb)


def _prep_shared(inp):
    out = {}

    Wqkv = np.asarray(inp["Wqkv"], np.float32).copy()  # [L, 3E, E]
    bqkv = np.asarray(inp["bqkv"], np.float32).copy()  # [L, 3E]
    # 1/sqrt(dh) is applied via the q copy-out scale; bq rows pre-divided.
    att_sc = 1.0 / np.sqrt(DH)
    bqkv[:, :E] *= att_sc

    # wqkv fp8 3-term blocks: [L, 6, 128, 4, 2, 2, 512] (Q g0-1, K g2-3, V g4-5)
    out["wqkv"] = np.stack([_block_dr2(Wqkv[l], 512) for l in range(L)])

    # wo fp16 head-pair lhsT: [L, 4(og), 128(hh*64+d), 8(hp), 256]
    Wo = np.asarray(inp["Wo"], np.float32)  # [L, E, E]
    woh = Wo.reshape(L, 4, 256, 8, 2, DH)   # l og o hp hh d
    woh = woh.transpose(0, 1, 4, 5, 3, 2)   # l og hh d hp o
    out["wo"] = _f16(woh.reshape(L, 4, 128, 8, 256))

    W1 = np.asarray(inp["W1"], np.float32)  # [L, F, E]
    out["w1"] = np.stack([_block_dr2(W1[l], 512) for l in range(L)])
    # w2 fp8 3-term: [L, 2(g), 4(cpg), 128, 4(cpi), 2, 2, 512]
    W2 = np.asarray(inp["W2"], np.float32)  # [L, E, F]
    w2b = np.stack([_block_dr2(W2[l], 512) for l in range(L)])
    # [L, 2, 128, 16, 2, 2, 512] -> split cp 16 -> (4, 4)
    w2b = w2b.reshape(L, 2, 128, 4, 4, 2, 2, 512).transpose(
        0, 1, 3, 2, 4, 5, 6, 7)
    out["w2"] = np.ascontiguousarray(w2b)

    # head GEMM commuted past the final LN:
    # logits = rstd*(x@Wp^T) - (mean*rstd)*rowsum(Wp) + (genW@lnf_b + gen_b)
    # with Wp = genW * lnf_s
    genW = np.asarray(inp["gen_W"], np.float32)  # [1200, E]
    lnf_s_v = np.asarray(inp["lnf_s"], np.float32)
    lnf_b_v = np.asarray(inp["lnf_b"], np.float32)
    Wp = genW * lnf_s_v[None, :]
    genW_pad = np.zeros((1280, E), np.float32)
    genW_pad[:1200] = Wp
    out["genw"] = _f16(_block_lhsT(genW_pad, 256))  # [5, 128, 8, 256]

    nws = np.zeros((1280,), np.float32)
    nws[:1200] = -Wp.sum(1)
    out["negw1sum_pp"] = np.ascontiguousarray(nws.reshape(NO, 128).T)
    gen_b = np.asarray(inp["gen_b"], np.float32)
    gbp = np.zeros((1280,), np.float32)
    gbp[:1200] = genW @ lnf_b_v + gen_b
    out["gen_b_pp"] = np.ascontiguousarray(gbp.reshape(NO, 128).T)  # [128, 10]

    out["bqkv_pp"] = _pp(bqkv[:, : 2 * E])  # [L, 128, 16] (q rows /8)
    out["bv_row"] = _f16(bqkv[:, 2 * E:].reshape(L, 1, E) * WSC)  # [L, 1, E]
    out["bo_pp"] = _pp(np.asarray(inp["bo"], np.float32))  # [L, 128, 8]
    out["b1_pp"] = _pp(np.asarray(inp["b1"], np.float32))  # [L, 128, 32]
    out["b2_pp"] = _pp(np.asarray(inp["b2"], np.float32))  # [L, 128, 8]

    ln_s = np.stack([np.asarray(inp["ln1_s"], np.float32),
                     np.asarray(inp["ln2_s"], np.float32)], 1)  # [L, 2, E]
    ln_b = np.stack([np.asarray(inp["ln1_b"], np.float32),
                     np.asarray(inp["ln2_b"], np.float32)], 1)
    out["ln_s_pp"] = _pp(ln_s)  # [L, 2, 128, 8]
    out["ln_b_pp"] = _pp(ln_b)
    out["lnf_s_pp"] = _pp(np.asarray(inp["lnf_s"], np.float32))  # [128, 8]
    out["lnf_b_pp"] = _pp(np.asarray(inp["lnf_b"], np.float32))

    out["valemb"] = _f16(np.asarray(inp["val_emb"], np.float32))   # [40, E]
    out["ringemb"] = _f16(np.asarray(inp["ring_emb"], np.float32))  # [30, E]

    # DR identity for mask add: idz[p, v, i, m] = 1 if i==v and p==m
    idz = np.zeros((128, 2, 2, 128), np.float32)
    for v in range(2):
        idz[:, v, v, :] = np.eye(128)
    out["idz"] = _f8(idz)
    out["ones_row"] = _f16(np.ones((1, S), np.float32))
    out["iota_col"] = np.ascontiguousarray(
        np.arange(128, dtype=np.float32).reshape(128, 1))
    out["ones_col"] = _f16(np.ones((128, 1), np.float32))
    return out


def _prep_percore(inp):
    """Per-core tensors: token rows + additive attention mask (fp8)."""
    val = np.asarray(inp["val_sequences"]).astype(np.int64)    # [B, S]
    ring = np.asarray(inp["ring_sequences"]).astype(np.int64)  # [B, S]
    dist = np.asarray(inp["distance_squares"]).astype(np.int64)  # [B, S, S]
    de = np.asarray(inp["dist_emb"], np.float32)  # [200, H]

    # mask[b, h, k, q] = de[dist[b, q, k], h] or NEG
    m = de[dist]                         # [B, S(q), S(k), H]
    m = m.transpose(0, 3, 2, 1)          # [B, H, k, q]
    kk = np.arange(S)
    causal = kk[:, None] <= kk[None, :]  # [k, q] keep where k <= q
    m = np.where(causal[None, None], m, NEG)
    padk = val == PAD_ID  # [B, S]
    m = np.where(padk[:, None, :, None], NEG, m)
    # -> [B, H, 128(p), 4(kc), S(q)] with k = kc*128 + p
    m = m.reshape(B, H, 4, 128, S).transpose(0, 1, 3, 2, 4)
    m = np.ascontiguousarray(m.reshape(B, H, 128, 4 * S).astype(F8))

    cores = []
    for b in range(B):
        cores.append({
            "mask": m[b],
            "valrow": np.ascontiguousarray(val[b].reshape(1, S).astype(F16)),
            "ringrow": np.ascontiguousarray(ring[b].reshape(1, S).astype(F16)),
        })
    return cores


# ----------------------------------------------------------------------------
# device program
# ----------------------------------------------------------------------------

def _declare(nc):
    d = {}

    def di(name, shape, dt):
        d[name] = nc.dram_tensor(name, list(shape), dt, kind="ExternalInput").ap()

    di("wqkv", (L, 6, 128, 4, 2, 2, 512), f8)
    di("wo", (L, 4, 128, 8, 256), f16)
    di("w1", (L, 8, 128, 4, 2, 2, 512), f8)
    di("w2", (L, 2, 4, 128, 4, 2, 2, 512), f8)
    di("genw", (5, 128, 8, 256), f16)
    di("gen_b_pp", (128, NO), f32)
    di("negw1sum_pp", (128, NO), f32)
    di("bqkv_pp", (L, 128, 16), f32)
    di("bv_row", (L, 1, E), f16)
    di("bo_pp", (L, 128, 8), f32)
    di("b1_pp", (L, 128, 32), f32)
    di("b2_pp", (L, 128, 8), f32)
    di("ln_s_pp", (L, 2, 128, 8), f32)
    di("ln_b_pp", (L, 2, 128, 8), f32)
    di("lnf_s_pp", (128, 8), f32)
    di("lnf_b_pp", (128, 8), f32)
    di("valemb", (VV, E), f16)
    di("ringemb", (VR, E), f16)
    di("idz", (128, 2, 2, 128), f8)
    di("ones_row", (1, S), f16)
    di("iota_col", (128, 1), f32)
    di("ones_col", (128, 1), f16)
    di("mask", (H, 128, 4 * S), f8)
    di("valrow", (1, S), f16)
    di("ringrow", (1, S), f16)
    d["logits"] = nc.dram_tensor(
        "logits", [NO, 128, S], f32, kind="ExternalOutput"
    ).ap()
    if os.environ.get("BG_DEBUG"):
        def do(name, shape, dt=f16):
            d[name] = nc.dram_tensor(name, list(shape), dt,
                                     kind="ExternalOutput").ap()
        do("dbg_h0", (128, S))
        do("dbg_hp0", (128, 2, S), f8)
        do("dbg_dhp0", (128, 2, S), f8)
        do("dbg_qk", (128, S))
        do("dbg_v", (128, 2, H, DH + 1), f8)
        do("dbg_at", (128, 2 * S), f8)
        do("dbg_ctx", (128, S))
        do("dbg_r1", (128, S))
        do("dbg_h1", (128, S))
        do("dbg_ff", (128, S))
        do("dbg_r2", (128, S))
    return d


def _emit(nc, tc, d, ctx):
    mm = nc.tensor.matmul

    cpool = ctx.enter_context(tc.tile_pool(name="cpool", bufs=1))
    maskpool = ctx.enter_context(tc.tile_pool(name="maskpool", bufs=3))
    wpool = ctx.enter_context(tc.tile_pool(name="wpool", bufs=3))
    wopool = ctx.enter_context(tc.tile_pool(name="wopool", bufs=2))
    hpool = ctx.enter_context(tc.tile_pool(name="hpool", bufs=17))
    hppool = ctx.enter_context(tc.tile_pool(name="hppool", bufs=8))
    dhppool = ctx.enter_context(tc.tile_pool(name="dhppool", bufs=8))
    qkpool = ctx.enter_context(tc.tile_pool(name="qkpool", bufs=16))
    vpool = ctx.enter_context(tc.tile_pool(name="vpool", bufs=3))
    atpool = ctx.enter_context(tc.tile_pool(name="atpool", bufs=5))
    ctxpool = ctx.enter_context(tc.tile_pool(name="ctxpool", bufs=9))
    ffpool = ctx.enter_context(tc.tile_pool(name="ffpool", bufs=17))
    tmppool = ctx.enter_context(tc.tile_pool(name="tmppool", bufs=6))
    smallf = ctx.enter_context(tc.tile_pool(name="smallf", bufs=4))
    smallb = ctx.enter_context(tc.tile_pool(name="smallb", bufs=3))
    recpool = ctx.enter_context(tc.tile_pool(name="recpool", bufs=3))
    outpool = ctx.enter_context(tc.tile_pool(name="outpool", bufs=2))
    pppool = ctx.enter_context(tc.tile_pool(name="pppool", bufs=4))

    ps_gemm = ctx.enter_context(tc.tile_pool(name="ps_gemm", bufs=4, space="PSUM"))
    ps_wide = ctx.enter_context(tc.tile_pool(name="ps_wide", bufs=2, space="PSUM"))

    hw = nc.sync  # HWDGE dma engine

    # --- constants -----------------------------------------------------------
    idz = cpool.tile([128, 2, 2, 128], f8)
    hw.dma_start(out=idz, in_=d["idz"])
    ones_row = cpool.tile([1, S], f16)
    hw.dma_start(out=ones_row, in_=d["ones_row"])
    iota_col = cpool.tile([128, 1], f32)
    hw.dma_start(out=iota_col, in_=d["iota_col"])
    ones_col = cpool.tile([128, 1], f16)
    hw.dma_start(out=ones_col, in_=d["ones_col"])
    valemb = cpool.tile([VV, E], f16)
    hw.dma_start(out=valemb, in_=d["valemb"])
    ringemb = cpool.tile([VR, E], f16)
    hw.dma_start(out=ringemb, in_=d["ringemb"])
    genb_pp = cpool.tile([128, NO], f32)
    hw.dma_start(out=genb_pp, in_=d["gen_b_pp"])
    negw1s = cpool.tile([128, NO], f32)
    hw.dma_start(out=negw1s, in_=d["negw1sum_pp"])
    eps_t = cpool.tile([128, 1], f32)
    nc.vector.memset(eps_t, 1e-5)
    actwarm = cpool.tile([1, 1], f32)
    lnf_s = cpool.tile([128, 8], f32)
    hw.dma_start(out=lnf_s, in_=d["lnf_s_pp"])
    lnf_b = cpool.tile([128, 8], f32)
    hw.dma_start(out=lnf_b, in_=d["lnf_b_pp"])

    # --- embedding -----------------------------------------------------------
    with nc.named_scope("embed"):
        valR = tmppool.tile([VV, S], f16, tag="ffh", bufs=3)
        nc.gpsimd.dma_start(out=valR, in_=d["valrow"].to_broadcast((VV, S)))
        ringR = tmppool.tile([VR, S], f16, tag="ffh", bufs=3)
        nc.gpsimd.dma_start(out=ringR, in_=d["ringrow"].to_broadcast((VR, S)))
        oh_val = tmppool.tile([VV, S], f16, tag="ffh", bufs=3)
        nc.vector.tensor_scalar(oh_val, valR, iota_col[:VV, :], None, OP.is_equal)
        oh_ring = tmppool.tile([VR, S], f16, tag="ffh", bufs=3)
        nc.vector.tensor_scalar(oh_ring, ringR, iota_col[:VR, :], None, OP.is_equal)

        h_t = []
        hp_t = [hppool.tile([128, 2, S], f8, tag="hp", name=f"emb_hp{j}")
                for j in range(NEP)]
        dhp_t = [dhppool.tile([128, 2, S], f8, tag="dhp", name=f"emb_dhp{j}")
                 for j in range(NEP)]
        for c in range(NE):
            ps = ps_gemm.tile([128, S], f32, tag="gemm")
            mm(ps, valemb[:, c * 128:(c + 1) * 128], oh_val, start=True, stop=False)
            mm(ps, ringemb[:, c * 128:(c + 1) * 128], oh_ring, start=False, stop=True)
            ht = hpool.tile([128, S], f16, tag="h")
            nc.scalar.activation(ht, ps, AF.Copy, scale=float(np.sqrt(E)))
            nc.scalar.activation(hp_t[c // 2][:, c % 2, :], ps, AF.Copy,
                                 scale=float(np.sqrt(E)))
            nc.vector.tensor_sub(dhp_t[c // 2][:, c % 2, :], ht,
                                 hp_t[c // 2][:, c % 2, :])
            h_t.append(ht)
        if "dbg_h0" in d:
            hw.dma_start(out=d["dbg_h0"], in_=h_t[0])
            hw.dma_start(out=d["dbg_hp0"], in_=hp_t[0])
            hw.dma_start(out=d["dbg_dhp0"], in_=dhp_t[0])

    # --- layers --------------------------------------------------------------
    env = dict(locals())
    for l in range(L):
        h_t, hp_t, dhp_t = _layer(nc, tc, d, l, h_t, hp_t, dhp_t, env)

    # --- final LN (stats only) + head on pre-LN x ----------------------------
    with nc.named_scope("final"):
        genw_sb = []
        for g in range(5):
            wt = wopool.tile([128, 8, 256], f16, tag="genw", bufs=3)
            hw.dma_start(out=wt, in_=d["genw"][g])
            genw_sb.append(wt)
        tmppool = env["tmppool"]; smallf = env["smallf"]; smallb = env["smallb"]
        recpool = env["recpool"]
        sums_r = ps_gemm.tile([1, S], f32, tag="gemm", name="flnsum_r")
        sums_q = ps_gemm.tile([1, S], f32, tag="gemm", name="flnsum_q")
        sq_t = []
        for c in range(NE):
            sq = tmppool.tile([128, S], f16, tag="sq", bufs=8)
            nc.vector.tensor_mul(sq, h_t[c], h_t[c])
            sq_t.append(sq)
        for c in range(NE):
            mm(sums_r, ones_col, h_t[c], start=(c == 0), stop=(c == NE - 1))
        for c in range(NE):
            mm(sums_q, ones_col, sq_t[c], start=(c == 0), stop=(c == NE - 1))
        s2 = smallf.tile([1, S], f32, tag="sf")
        nc.scalar.activation(s2, sums_r, AF.Square)
        varE = smallf.tile([1, S], f32, tag="sf")
        nc.vector.scalar_tensor_tensor(varE, s2, -1.0 / E, sums_q,
                                       OP.mult, OP.add)
        std = smallf.tile([1, S], f32, tag="sf")
        nc.scalar.activation(std, varE, AF.Sqrt, bias=eps_t[:1, :],
                             scale=1.0 / E)
        rstd = smallf.tile([1, S], f32, tag="sf")
        nc.vector.reciprocal_approx_fast(out=rstd, in_=std)
        rstd_b = smallb.tile([1, S], f16, tag="sb")
        nc.vector.tensor_copy(rstd_b, rstd)
        mrstd_b = smallb.tile([1, S], f16, tag="sb")
        # mean * rstd
        nc.vector.scalar_tensor_tensor(mrstd_b, sums_r, 1.0 / E, rstd,
                                       OP.mult, OP.mult)
        rstdR = recpool.tile([128, S], f16, tag="rec", name="flnrstdR")
        nc.gpsimd.partition_broadcast(rstdR, rstd_b, channels=128)
        mrstdR = recpool.tile([128, S], f16, tag="rec", name="flnmrstdR")
        nc.gpsimd.partition_broadcast(mrstdR, mrstd_b, channels=128)
        for mt in range(NO):
            g, mi = divmod(mt, 2)
            ps = ps_gemm.tile([128, S], f32, tag="gemm")
            for c in range(NE):
                mm(ps, genw_sb[g][:, c, mi * 128:(mi + 1) * 128], h_t[c],
                   start=(c == 0), stop=(c == NE - 1))
            gt = tmppool.tile([128, S], f16, tag="tmp", bufs=4,
                              name=f"gt{mt}")
            nc.vector.tensor_mul(gt, ps, rstdR)
            g2 = tmppool.tile([128, S], f16, tag="ffh", bufs=3,
                              name=f"g2{mt}")
            nc.vector.scalar_tensor_tensor(g2, mrstdR, negw1s[:, mt:mt + 1],
                                           gt, OP.mult, OP.add)
            ot = outpool.tile([128, S], f32, tag="f32out")
            nc.scalar.activation(ot, g2, AF.Identity,
                                 bias=genb_pp[:, mt:mt + 1])
            hw.dma_start(out=d["logits"][mt], in_=ot)


def _layernorm(nc, r_t, s_pp, b_pp, env, want_fp8=True, tagsuf=""):
    """r_t: 8 fp16 [128, S] tiles -> (fp16 tiles, fp8 pair tiles, fp8
    residual pair tiles).  Chunk sums tree-reduce on DVE, partition
    reduction via two ones-column matmuls per statistic."""
    mm = nc.tensor.matmul
    tmppool = env["tmppool"]; smallf = env["smallf"]; smallb = env["smallb"]
    recpool = env["recpool"]; hpool = env["hpool"]
    hppool = env["hppool"]; dhppool = env["dhppool"]
    ps_gemm = env["ps_gemm"]; ones_col = env["ones_col"]; eps_t = env["eps_t"]

    sums_r = ps_gemm.tile([1, S], f32, tag="gemm", name="lnsum_r" + tagsuf)
    sums_q = ps_gemm.tile([1, S], f32, tag="gemm", name="lnsum_q" + tagsuf)
    sq_t = []
    for c in range(NE):
        sq = tmppool.tile([128, S], f16, tag="sq", bufs=8)
        nc.vector.tensor_mul(sq, r_t[c], r_t[c])
        sq_t.append(sq)
    for c in range(NE):
        mm(sums_r, ones_col, r_t[c], start=(c == 0), stop=(c == NE - 1))
    for c in range(NE):
        mm(sums_q, ones_col, sq_t[c], start=(c == 0), stop=(c == NE - 1))

    # mean broadcast early; r centered while the var chain runs
    mean_b = smallb.tile([1, S], f16, tag="sb")
    nc.vector.tensor_scalar(mean_b, sums_r, 1.0 / E, None, OP.mult)
    meanR = recpool.tile([128, S], f16, tag="rec", name="lnmeanR" + tagsuf)
    nc.gpsimd.partition_broadcast(meanR, mean_b, channels=128)
    t2_t = []
    for c in range(NE):
        t2 = tmppool.tile([128, S], f16, tag="sq", bufs=8)
        eng = nc.vector if c < 6 else nc.gpsimd
        eng.tensor_sub(t2, r_t[c], meanR)
        t2_t.append(t2)

    s2 = smallf.tile([1, S], f32, tag="sf")
    nc.scalar.activation(s2, sums_r, AF.Square)
    varE = smallf.tile([1, S], f32, tag="sf")
    # varE = sumsq - s2/E  (= E * var)
    nc.vector.scalar_tensor_tensor(varE, s2, -1.0 / E, sums_q, OP.mult, OP.add)
    std = smallf.tile([1, S], f32, tag="sf")
    nc.scalar.activation(std, varE, AF.Sqrt, bias=eps_t[:1, :], scale=1.0 / E)
    rstd = smallf.tile([1, S], f32, tag="sf")
    nc.vector.reciprocal_approx_fast(out=rstd, in_=std)
    rstd_b = smallb.tile([1, S], f16, tag="sb")
    nc.vector.tensor_copy(rstd_b, rstd)
    rstdR = recpool.tile([128, S], f16, tag="rec", name="lnrstdR" + tagsuf)
    nc.gpsimd.partition_broadcast(rstdR, rstd_b, channels=128)

    out_t = []
    outp_t = [hppool.tile([128, 2, S], f8, tag="hp", name=f"hp{tagsuf}_{j}")
              for j in range(NEP)] if want_fp8 else None
    outdp_t = [dhppool.tile([128, 2, S], f8, tag="dhp", name=f"dhp{tagsuf}_{j}")
               for j in range(NEP)] if want_fp8 else None
    for c in range(NE):
        eng = nc.vector if c < 6 else nc.gpsimd
        t1 = tmppool.tile([128, S], f16, tag="tmp", bufs=4)
        eng.tensor_mul(t1, t2_t[c], rstdR)
        ht = hpool.tile([128, S], f16, tag="h")
        eng.tensor_scalar(ht, t1, s_pp[:, c:c + 1], b_pp[:, c:c + 1],
                          OP.mult, OP.add)
        out_t.append(ht)
        if want_fp8:
            nc.scalar.activation(outp_t[c // 2][:, c % 2, :], t1, AF.Identity,
                                 bias=b_pp[:, c:c + 1], scale=s_pp[:, c:c + 1])
            nc.vector.tensor_sub(outdp_t[c // 2][:, c % 2, :], ht,
                                 outp_t[c // 2][:, c % 2, :])
    return out_t, outp_t, outdp_t


def _layer(nc, tc, d, l, h_t, hp_t, dhp_t, env):
    mm = nc.tensor.matmul
    hw = nc.sync
    wpool = env["wpool"]; wopool = env["wopool"]; hpool = env["hpool"]
    qkpool = env["qkpool"]; vpool = env["vpool"]; atpool = env["atpool"]
    ctxpool = env["ctxpool"]; ffpool = env["ffpool"]; tmppool = env["tmppool"]
    smallf = env["smallf"]; recpool = env["recpool"]; pppool = env["pppool"]
    maskpool = env["maskpool"]
    ps_gemm = env["ps_gemm"]; ps_wide = env["ps_wide"]
    ones_row = env["ones_row"]; idz = env["idz"]

    # per-layer small params
    bqkv_pp = pppool.tile([128, 16], f32, tag="pp16")
    hw.dma_start(out=bqkv_pp, in_=d["bqkv_pp"][l])
    bv_row = pppool.tile([1, E], f16, tag="bvrow", bufs=1)
    hw.dma_start(out=bv_row, in_=d["bv_row"][l])
    bo_pp = pppool.tile([128, 8], f32, tag="pp8")
    hw.dma_start(out=bo_pp, in_=d["bo_pp"][l])
    b1_pp = pppool.tile([128, 32], f32, tag="pp32")
    hw.dma_start(out=b1_pp, in_=d["b1_pp"][l])
    b2_pp = pppool.tile([128, 8], f32, tag="pp8")
    hw.dma_start(out=b2_pp, in_=d["b2_pp"][l])
    ln_s = [pppool.tile([128, 8], f32, tag="pp8", name=f"lns{l}_{i}")
            for i in range(2)]
    ln_b = [pppool.tile([128, 8], f32, tag="pp8", name=f"lnb{l}_{i}")
            for i in range(2)]
    for i in range(2):
        hw.dma_start(out=ln_s[i], in_=d["ln_s_pp"][l, i])
        hw.dma_start(out=ln_b[i], in_=d["ln_b_pp"][l, i])

    def gemm3(ps, wt, mi, xp, dxp, ncp=4):
        """3-term compensated fp8 DR accumulation into ps."""
        for cp in range(ncp):
            w8 = wt[:, cp, :, 0, mi * 128:(mi + 1) * 128]
            dw8 = wt[:, cp, :, 1, mi * 128:(mi + 1) * 128]
            mm(ps, w8, xp[cp], start=(cp == 0), stop=False, perf_mode=DR)
            mm(ps, dw8, xp[cp], start=False, stop=False, perf_mode=DR)
            mm(ps, w8, dxp[cp], start=False, stop=(cp == ncp - 1), perf_mode=DR)

    # --- QKV -----------------------------------------------------------------
    with nc.named_scope(f"L{l}_qkv"):
        qk_t = []  # 16 fp16 tiles: q 0..7, k 8..15
        for g in range(4):  # Q, K feature-major, fp8 3-term
            wt = wpool.tile([128, 4, 2, 2, 512], f8, tag="w")
            hw.dma_start(out=wt, in_=d["wqkv"][l, g])
            for mi in range(4):
                mt = g * 4 + mi
                ps = ps_gemm.tile([128, S], f32, tag="gemm")
                gemm3(ps, wt, mi, hp_t, dhp_t)
                qk = qkpool.tile([128, S], f16, tag="qk")
                sc = DEQ / np.sqrt(DH) if g < 2 else DEQ
                nc.scalar.activation(qk, ps, AF.Identity,
                                     bias=bqkv_pp[:, mt:mt + 1], scale=float(sc))
                if l == 0 and mt == 0 and "dbg_qk" in d:
                    hw.dma_start(out=d["dbg_qk"], in_=qk)
                qk_t.append(qk)
        # V token-major fp8 pair tiles [128, 2, H, DH+1] + fp8 residual
        v_t = []
        dv_t = []
        for kcp in range(2):
            vt = vpool.tile([128, 2, H, DH + 1], f8, tag="v")
            nc.vector.memset(vt[:, :, :, DH:DH + 1], 1.0)
            v_t.append(vt)
            dvt = vpool.tile([128, 2, H, DH + 1], f8, tag="dv", bufs=2)
            nc.vector.memset(dvt[:, :, :, DH:DH + 1], 0.0)
            dv_t.append(dvt)
        for g in range(2):
            wt = wpool.tile([128, 4, 2, 2, 512], f8, tag="w")
            hw.dma_start(out=wt, in_=d["wqkv"][l, 4 + g])
            for n in range(4):
                ps = ps_gemm.tile([128, S], f32, tag="gemm")
                for cp in range(4):
                    xs = hp_t[cp][:, :, n * 128:(n + 1) * 128]
                    dxs = dhp_t[cp][:, :, n * 128:(n + 1) * 128]
                    w8 = wt[:, cp, :, 0, :]
                    dw8 = wt[:, cp, :, 1, :]
                    mm(ps, xs, w8, start=(cp == 0), stop=False, perf_mode=DR)
                    mm(ps, xs, dw8, start=False, stop=False, perf_mode=DR)
                    mm(ps, dxs, w8, start=False, stop=False, perf_mode=DR)
                mm(ps, ones_row[:, :128], bv_row[:, g * 512:(g + 1) * 512],
                   start=False, stop=True)
                v8s = v_t[n // 2][:, n % 2, g * 8:(g + 1) * 8, 0:DH]
                nc.scalar.activation(
                    v8s, ps.rearrange("p (a b) -> p a b", a=8),
                    AF.Copy, scale=DEQ)
                nc.vector.scalar_tensor_tensor(
                    dv_t[n // 2][:, n % 2, g * 8:(g + 1) * 8, 0:DH],
                    ps.rearrange("p (a b) -> p a b", a=8), DEQ, v8s,
                    OP.mult, OP.subtract)
        if l == 0 and "dbg_v" in d:
            hw.dma_start(out=d["dbg_v"], in_=v_t[0])

    # --- attention ------------------------------------------------------------
    with nc.named_scope(f"L{l}_attn"):
        ctx_t = [ctxpool.tile([128, S], f16, tag="ctx", name=f"cx{l}_{j}")
                 for j in range(8)]
        at_q = {}

        def emit_scores(h):
            qt = qk_t[h // 2]
            kt = qk_t[8 + h // 2]
            r0 = (h % 2) * DH
            mt_ = maskpool.tile([128, 4 * S], f8, tag="mask", name=f"mk{l}_{h}")
            hw.dma_start(out=mt_, in_=d["mask"][h])
            for kcp in range(2):
                sps = ps_wide.tile([128, 2 * S], f32, tag="wide",
                                   name=f"s{l}_{h}_{kcp}")
                for kci in range(2):
                    kc = kcp * 2 + kci
                    qlo = kc * 128
                    # mask first (start=True fills masked region with -240)
                    mm(sps[:, kci * S:(kci + 1) * S], idz[:, kci],
                       mt_[:, kcp * 2 * S:(kcp + 1) * 2 * S].rearrange(
                           "p (two n) -> p two n", two=2),
                       start=True, stop=False, perf_mode=DR)
                    # causal-restricted scores accumulate
                    mm(sps[:, kci * S + qlo:(kci + 1) * S],
                       kt[r0:r0 + DH, kc * 128:(kc + 1) * 128],
                       qt[r0:r0 + DH, qlo:], start=False, stop=True)
                at = atpool.tile([128, 2 * S], f8, tag="at",
                                 name=f"a{l}_{h}_{kcp}")
                nc.scalar.activation(at, sps, AF.Exp)
                if l == 0 and h == 0 and kcp == 0 and "dbg_at" in d:
                    hw.dma_start(out=d["dbg_at"], in_=at)
                at_q[(h, kcp)] = at

        def emit_av(h):
            cps = ps_gemm.tile([128, S], f32, tag="gemm", name=f"c{l}_{h}")
            for kcp in range(2):
                at = at_q.pop((h, kcp))
                atr = at.rearrange("p (two n) -> p two n", two=2)
                mm(cps[0:DH + 1, :], v_t[kcp][:, :, h, :], atr,
                   start=(kcp == 0), stop=False, perf_mode=DR)
                mm(cps[0:DH + 1, :], dv_t[kcp][:, :, h, :], atr,
                   start=False, stop=(kcp == 1), perf_mode=DR)
            srow = smallf.tile([1, S], f32, tag="sf", name=f"sr{l}_{h}")
            nc.vector.tensor_copy(srow, cps[DH:DH + 1, :])
            rec = smallf.tile([1, S], f32, tag="sf", name=f"re{l}_{h}")
            nc.vector.reciprocal_approx_fast(out=rec, in_=srow)
            recR = recpool.tile([DH, S], f32, tag="recf", name=f"rr{l}_{h}")
            nc.gpsimd.partition_broadcast(recR, rec, channels=DH)
            hh = h % 2
            nc.vector.tensor_mul(ctx_t[h // 2][hh * DH:(hh + 1) * DH, :],
                                 cps[0:DH, :], recR)

        emit_scores(0)
        for h in range(1, H):
            emit_scores(h)
            emit_av(h - 1)
        emit_av(H - 1)

        # out-proj fp16 (K = 8 head-pair chunks) + residual
        r1_t = []
        for og in range(4):
            wt = wopool.tile([128, 8, 256], f16, tag="wo")
            hw.dma_start(out=wt, in_=d["wo"][l, og])
            for mi in range(2):
                mt = og * 2 + mi
                ps = ps_gemm.tile([128, S], f32, tag="gemm")
                for hp in range(8):
                    mm(ps, wt[:, hp, mi * 128:(mi + 1) * 128], ctx_t[hp],
                       start=(hp == 0), stop=(hp == 7))
                r1 = hpool.tile([128, S], f16, tag="h")
                nc.vector.scalar_tensor_tensor(r1, ps, bo_pp[:, mt:mt + 1],
                                               h_t[mt], OP.add, OP.add)
                if l == 0 and mt == 0 and "dbg_r1" in d:
                    hw.dma_start(out=d["dbg_ctx"], in_=ctx_t[0])
                    hw.dma_start(out=d["dbg_r1"], in_=r1)
                r1_t.append(r1)

    with nc.named_scope(f"L{l}_ln1"):
        h1_t, h1p_t, dh1p_t = _layernorm(nc, r1_t, ln_s[0], ln_b[0], env,
                                         tagsuf=f"1_{l}")
        if l == 0 and "dbg_h1" in d:
            hw.dma_start(out=d["dbg_h1"], in_=h1_t[0])

    # --- FFN -----------------------------------------------------------------
    with nc.named_scope(f"L{l}_ffn"):
        ffp_t = [ffpool.tile([128, 2, S], f8, tag="ff", name=f"ff{l}_{j}")
                 for j in range(NF // 2)]
        dffp_t = [ffpool.tile([128, 2, S], f8, tag="dff", name=f"dff{l}_{j}")
                  for j in range(NF // 2)]
        for g in range(8):
            wt = wpool.tile([128, 4, 2, 2, 512], f8, tag="w")
            hw.dma_start(out=wt, in_=d["w1"][l, g])
            for mi in range(4):
                mt = g * 4 + mi
                ps = ps_gemm.tile([128, S], f32, tag="gemm")
                gemm3(ps, wt, mi, h1p_t, dh1p_t)
                ft = tmppool.tile([128, S], f16, tag="ffh", bufs=3,
                                  name=f"ffh{l}_{mt}")
                nc.scalar.activation(ft, ps, AF.Gelu,
                                     bias=b1_pp[:, mt:mt + 1], scale=DEQ)
                f8s = ffp_t[mt // 2][:, mt % 2, :]
                nc.scalar.activation(f8s, ft, AF.Copy)
                nc.vector.tensor_sub(dffp_t[mt // 2][:, mt % 2, :], ft, f8s)
        r2_t = [None] * NE
        for g in range(2):
            pss = [ps_gemm.tile([128, S], f32, tag="gemm",
                                name=f"ff2ps{l}_{g}_{i}") for i in range(4)]
            for cpg in range(4):
                wt = wpool.tile([128, 4, 2, 2, 512], f8, tag="w",
                                name=f"w2t{l}_{g}_{cpg}")
                hw.dma_start(out=wt, in_=d["w2"][l, g, cpg])
                for cpi in range(4):
                    cp = cpg * 4 + cpi
                    for mi in range(4):
                        w8 = wt[:, cpi, :, 0, mi * 128:(mi + 1) * 128]
                        dw8 = wt[:, cpi, :, 1, mi * 128:(mi + 1) * 128]
                        mm(pss[mi], w8, ffp_t[cp], start=(cp == 0),
                           stop=False, perf_mode=DR)
                        mm(pss[mi], dw8, ffp_t[cp], start=False,
                           stop=False, perf_mode=DR)
                        mm(pss[mi], w8, dffp_t[cp], start=False,
                           stop=(cp == 15), perf_mode=DR)
            for mi in range(4):
                mt = g * 4 + mi
                f2o = tmppool.tile([128, S], f16, tag="ffh", bufs=3,
                                   name=f"f2o{l}_{mt}")
                nc.scalar.activation(f2o, pss[mi], AF.Identity,
                                     bias=b2_pp[:, mt:mt + 1], scale=DEQ)
                r2 = hpool.tile([128, S], f16, tag="h")
                nc.vector.tensor_add(r2, f2o, h1_t[mt])
                if l == 0 and mt == 0 and "dbg_r2" in d:
                    hw.dma_start(out=d["dbg_r2"], in_=r2)
                r2_t[mt] = r2

    with nc.named_scope(f"L{l}_ln2"):
        h2_t, h2p_t, dh2p_t = _layernorm(nc, r2_t, ln_s[1], ln_b[1], env,
                                         tagsuf=f"2_{l}")
    return h2_t, h2p_t, dh2p_t


def _build():
    if "nc" in _CACHE:
        return _CACHE["nc"]
    from contextlib import ExitStack

    nc = bacc.Bacc("TRN2", debug=False)
    d = _declare(nc)
    with tile.TileContext(nc) as tc:
        with ExitStack() as ctx:
            _emit(nc, tc, d, ctx)
    nc.compile()
    _CACHE["nc"] = nc
    return nc


def kernel_internal(inputs, trace=False, trace_kwargs=None):
    shared = _prep_shared(inputs)
    cores = _prep_percore(inputs)
    nc = _build()
    in_maps = []
    for b in range(B):
        m = dict(shared)
        m.update(cores[b])
        in_maps.append(m)
    res = run_bass_kernel_spmd(
        nc, in_maps, core_ids=list(range(B)), trace=trace,
        **(trace_kwargs or {}),
    )
    outs = []
    for b in range(B):
        lo = res.results[b]["logits"]  # [10, 128, 512]
        lo = lo.reshape(NO * 128, S)[:VV * VR].T  # [512, 1200]
        outs.append(lo)
    out = np.stack(outs).astype(np.float32)  # [B, S, 1200]
    return out, res


def kernel(**inputs):
    out, _ = kernel_internal(inputs)
    return out
